# revision 17
# baseline (speedup 1.0000x reference)
"""Trainium2 Bass kernel for nn_BaselineModel_35175782154746 (dense transformer
block with SiLU attention + relative-position bias).

Sharding: 8 NeuronCores = 4 batches x 2 head-groups (8 heads each).
Each core computes, for its (batch b, head-group g):
    U, Q, K, V projections (columns g*1024:(g+1)*1024 of Wu/Wq/Wk/Wv),
    SiLU attention with rel-pos bias for its 8 heads,
    gated = out * U, partial = gated @ Wf2[g*1024:(g+1)*1024, :].
Host reduces: out[b] = partial[2b] + partial[2b+1] + bf2.

All matmuls run with bf16 operands (fp32 PSUM accumulation). The causal
build exploits causality exactly: score / bias / silu / AV work is trimmed
to queries >= key-block start (the above-diagonal wedge inside the
diagonal 128-block is masked via the -1e5 entries baked into the shifted
bias table, which silu maps to an exact 0).

Scheduling (causal build): projections run in 4-head PSUM rounds
alternating two 4-bank groups so activation drain overlaps the next
round's matmuls; weight k-tiles persist across both query halves (each
weight byte is fetched once); the V projection, attention ih=0 and f2 are
emitted as one interleaved stream so the TensorEngine never idles (idle
gaps also drop it out of its max p-state). Bias-add STTs and copies
alternate between the Vector and GpSimd engines.
"""

import sys
import os

for _p in ("/root/.axon_site/_ro/trn_rl_repo", "/opt/trn_rl_repo"):
    if os.path.isdir(_p) and _p not in sys.path:
        sys.path.append(_p)

import numpy as np

import concourse.bass as bass
import concourse.mybir as mybir
import concourse.tile as tile
from concourse import bacc
from concourse.bass_utils import run_bass_kernel_spmd

B, S, H, NH, MAXLEN = 4, 1024, 2048, 16, 1024
HD = H // NH            # 128
NHL = 8                 # heads per core (local)
HGRP = 2                # head groups
NCORES = 8
KT16 = H // 128         # 16 k-tiles for the H contraction
SCALE = float(HD) ** -0.5

f32 = mybir.dt.float32
f32r = mybir.dt.float32r
bf16 = mybir.dt.bfloat16
f8 = mybir.dt.float8e4
SILU = mybir.ActivationFunctionType.Silu
MULT = mybir.AluOpType.mult
ADD = mybir.AluOpType.add
DR = mybir.MatmulPerfMode.DoubleRow
FP8_SC = False          # fp8 DoubleRow score matmuls (Q/K repacked [64,2,.])
FP8_AV = True           # fp8 DoubleRow AV matmuls (V + attention probs fp8)

TRACE = False
LAST_EXEC_NS = None
LAST_RES = None
MM_DT = "bf16"          # "bf16" or "f32r" matmul operand dtype
_CACHE = {}


def _build_v2(mm_dt=None):
    """Causal-only build with interleaved emission."""
    mmdt = {"bf16": bf16, "f32r": f32r}[mm_dt or MM_DT]
    nc = bacc.Bacc("TRN2", target_bir_lowering=False, debug=False,
                   num_devices=NCORES)

    def din(name, shape, dt=f32):
        return nc.dram_tensor(name, shape, dt, kind="ExternalInput").ap()

    qT = din("qT", [H, S], mmdt)
    kT = din("kT", [H, S], mmdt)
    vT = din("vT", [H, S], mmdt)
    wq = din("wq", [H, NHL * HD], mmdt)
    wk = din("wk", [H, NHL * HD], mmdt)
    wv = din("wv", [H, NHL * HD], mmdt)
    wu = din("wu", [H, NHL * HD], mmdt)
    wf2 = din("wf2", [NHL * HD, H], mmdt)
    bq = din("bq", [128, NHL])
    bk = din("bk", [128, NHL])
    bu = din("bu", [128, NHL])
    bv = din("bv", [1, NHL * HD], mmdt)
    ones1 = din("ones1", [1, 128], mmdt)
    atab = din("atab", [NHL, 128, 2047], bf16)
    out = nc.dram_tensor("out", [S, H], f32, kind="ExternalOutput").ap()

    with tile.TileContext(nc) as tc:
        constp = tc.alloc_tile_pool(name="const", bufs=1)
        bigres = tc.alloc_tile_pool(name="bigres", bufs=1)
        attnp = tc.alloc_tile_pool(name="attnp", bufs=6 if FP8_AV else 12)
        attscr = tc.alloc_tile_pool(name="attscr", bufs=4)
        atabp = tc.alloc_tile_pool(name="atabp", bufs=1)
        qresp = tc.alloc_tile_pool(name="qresp", bufs=1)
        winp = tc.alloc_tile_pool(name="winp", bufs=8)
        kresp = tc.alloc_tile_pool(name="kresp", bufs=1)

        bq_t = constp.tile([128, NHL], f32, tag="bq")
        bk_t = constp.tile([128, NHL], f32, tag="bk")
        bu_t = constp.tile([128, NHL], f32, tag="bu")
        bv_t = constp.tile([1, NHL * HD], mmdt, tag="bv")
        ones_t = constp.tile([1, 128], mmdt, tag="ones1")

        qk_dt = f8 if FP8_SC else mmdt
        av_dt = f8 if FP8_AV else mmdt
        UT = bigres.tile([128, NHL, S], bf16, tag="UT")
        QT = bigres.tile([128, NHL, S], qk_dt, tag="QT")
        KTt = bigres.tile([128, NHL, S], qk_dt, tag="KT")
        V = bigres.tile([128, NHL, S], av_dt, tag="V")
        gatedT = bigres.tile([128, NHL, S], mmdt, tag="gatedT")
        if FP8_SC:
            # [64, 2, h, s]: head-dim split into 2 k-tiles of 64 partitions
            # for the DoubleRow score matmul; filled by SBUF-SBUF repack.
            Q8 = bigres.tile([64, 2, NHL, S], f8, tag="Q8")
            K8 = bigres.tile([64, 2, NHL, S], f8, tag="K8")

        qres = qresp.tile([128, KT16, S], mmdt, tag="qres")
        kres = kresp.tile([128, KT16, S], mmdt, tag="kres")
        # vres shares qres's slot: qres's last read is the Q phase and the
        # vres load lands during K.
        vres = qresp.tile([128, KT16, S], mmdt, tag="qres", name="vres")

        # ---- input DMAs: first q k-tile first so U can start ASAP ----
        nc.sync.dma_start(qres[:, 0, :], qT[0:128, :])
        nc.sync.dma_start(bu_t[:], bu[:])
        nc.sync.dma_start(bq_t[:], bq[:])
        nc.sync.dma_start(bk_t[:], bk[:])
        nc.sync.dma_start(bv_t[:], bv[:])
        nc.sync.dma_start(ones_t[:], ones1[:])
        for k in range(1, KT16):
            nc.sync.dma_start(qres[:, k, :], qT[k * 128:(k + 1) * 128, :])
        for k in range(KT16):
            nc.sync.dma_start(kres[:, k, :], kT[k * 128:(k + 1) * 128, :])
        for k in range(KT16):
            nc.sync.dma_start(vres[:, k, :], vT[k * 128:(k + 1) * 128, :])
        at_tiles = [atabp.tile([128, 2047], bf16, tag=f"atab{h}",
                               name=f"atab{h}")
                    for h in range(NHL)]
        for h in range(NHL):
            nc.sync.dma_start(at_tiles[h][:], atab[h])

        # ================= U, Q, K projections =================
        # 4-head rounds, two alternating 4-bank PSUM groups; weight k-tiles
        # [128, 512] persist across both query halves of a round-pair.
        pproj = tc.alloc_tile_pool(name="pproj", bufs=1, space="PSUM")
        rnd = [0]
        wtile = {}

        def proj_round(wdram, xres, btile, outtile, half, ih):
            grp = ((rnd[0] + 1) % 2) * 4
            rnd[0] += 1
            ps = [pproj.tile([128, 512], f32, tag=f"pp{grp + j}",
                             name=f"pp{rnd[0]}_{j}")
                  for j in range(4)]
            for k in range(KT16):
                wt = winp.tile([128, 512], mmdt, tag="win",
                               name=f"w{k}_{rnd[0]}")
                qeng = nc.scalar if (rnd[0] == 1 and k % 2) else nc.gpsimd
                qeng.dma_start(
                    wt[:], wdram[k * 128:(k + 1) * 128,
                                 half * 512:(half + 1) * 512])
                for j in range(4):
                    nc.tensor.matmul(
                        ps[j][:],
                        lhsT=wt[:, j * HD:(j + 1) * HD],
                        rhs=xres[:, k, ih * 512:(ih + 1) * 512],
                        start=(k == 0), stop=(k == KT16 - 1))
            for j in range(4):
                h = half * 4 + j
                nc.scalar.activation(
                    outtile[:, h, ih * 512:(ih + 1) * 512],
                    ps[j][:], SILU, bias=btile[:, h:h + 1])

        def proj_all(wdram, xres, btile, outtile):
            for half in range(2):
                for ih in range(2):
                    proj_round(wdram, xres, btile, outtile, half, ih)

        proj_all(wu, qres, bu_t, UT)
        proj_all(wq, qres, bq_t, QT)
        if FP8_SC:
            # repack Q to [64, 2, h, s] during the K projection
            nc.scalar.dma_start(Q8[:, 0], QT[0:64])
            nc.scalar.dma_start(Q8[:, 1], QT[64:128])
        proj_all(wk, kres, bk_t, KTt)
        if FP8_SC:
            nc.scalar.dma_start(K8[:, 0], KTt[0:64])
            nc.scalar.dma_start(K8[:, 1], KTt[64:128])

        pproj.release()
        kresp.release()

        # ============ V projection + attention + f2: one stream ===========
        pssc = tc.alloc_tile_pool(name="pssc", bufs=3, space="PSUM")
        psav = tc.alloc_tile_pool(name="psav", bufs=1, space="PSUM")
        psv = tc.alloc_tile_pool(name="psv", bufs=1, space="PSUM")

        scn = [0]
        attq = {}
        avps = {}

        def emit_sc(h, ih, jbs):
            """Score + bias + silu for (h, ih, jb in jbs), trimmed to the
            causal query range. With FP8_AV the trim is pair-granular (the
            odd block's extra wedge is silu(-1e5) = 0, needed since the AV
            matmul consumes both pair planes over the same column range)."""
            at = at_tiles[h]
            for jb in jbs:
                if FP8_AV:
                    q0 = max(0, (jb // 2) * 256 - ih * 512)
                else:
                    q0 = max(0, jb * 128 - ih * 512)
                scp = pssc.tile([128, 512], f32, tag="sc",
                                name=f"sc{h}_{ih}_{jb}")
                if FP8_SC:
                    nc.tensor.matmul(
                        scp[:, q0:512],
                        lhsT=K8[:, :, h, jb * 128:(jb + 1) * 128],
                        rhs=Q8[:, :, h, ih * 512 + q0:(ih + 1) * 512],
                        start=True, stop=True, perf_mode=DR)
                else:
                    nc.tensor.matmul(
                        scp[:, q0:512],
                        lhsT=KTt[:, h, jb * 128:(jb + 1) * 128],
                        rhs=QT[:, h, ih * 512 + q0:(ih + 1) * 512],
                        start=True, stop=True)
                d0 = ih * 512 - jb * 128 + MAXLEN - 1
                if FP8_AV:
                    if jb % 2 == 0:
                        attq[(h, ih, jb // 2)] = attnp.tile(
                            [128, 2, 512], f8, tag="att",
                            name=f"at{h}_{ih}_{jb // 2}")
                    pair = attq[(h, ih, jb // 2)]
                    scr = attscr.tile([128, 512], bf16, tag="scr",
                                      name=f"scr{h}_{ih}_{jb}")
                    nc.vector.scalar_tensor_tensor(
                        scr[:, q0:512], scp[:, q0:512], SCALE,
                        at[:, d0 + q0:d0 + 512], op0=MULT, op1=ADD)
                    nc.scalar.activation(pair[:, jb % 2, q0:512],
                                         scr[:, q0:512], SILU)
                else:
                    att = attnp.tile([128, 512], mmdt, tag="att",
                                     name=f"at{h}_{ih}_{jb}")
                    nc.vector.scalar_tensor_tensor(
                        att[:, q0:512], scp[:, q0:512], SCALE,
                        at[:, d0 + q0:d0 + 512], op0=MULT, op1=ADD)
                    nc.scalar.activation(att[:, q0:512], att[:, q0:512],
                                         SILU)
                    attq[(h, ih, jb)] = att

        def emit_av(h, ih, jbs, last_jb):
            avp = avps[(h, ih)]
            if FP8_AV:
                for p in jbs:
                    q0 = max(0, p * 256 - ih * 512)
                    nc.tensor.matmul(
                        avp[:, q0:512],
                        lhsT=V[:, 2 * p:2 * p + 2, h * HD:(h + 1) * HD],
                        rhs=attq.pop((h, ih, p))[:, :, q0:512],
                        start=(p == 0), stop=(p == last_jb),
                        perf_mode=DR, skip_group_check=True)
            else:
                for jb in jbs:
                    q0 = max(0, jb * 128 - ih * 512)
                    nc.tensor.matmul(
                        avp[:, q0:512],
                        lhsT=V[:, jb, h * HD:(h + 1) * HD],
                        rhs=attq.pop((h, ih, jb))[:, q0:512],
                        start=(jb == 0), stop=(jb == last_jb),
                        skip_group_check=True)

        def new_avp(h, ih):
            avps[(h, ih)] = psav.tile([128, 512], f32, tag="av",
                                      name=f"av{h}_{ih}")

        def emit_gated(h, ih):
            nc.vector.scalar_tensor_tensor(
                gatedT[:, h, ih * 512:(ih + 1) * 512],
                avps.pop((h, ih))[:], 1.0,
                UT[:, h, ih * 512:(ih + 1) * 512],
                op0=MULT, op1=MULT)

        # V rounds: (ch, sbh) in order (0,0) (1,0) (1,1) (0,1) so that AV
        # of ih=0 (key blocks 0-3 = sbh 0) unlocks after two rounds and
        # the ch1 weight tiles are reused across adjacent rounds. wv tiles
        # reuse the projection weight tags (their last reads precede V).
        vps = {}
        wvtile = {}

        def v_chunk(ch, sbh, ks, open_, close, load):
            if open_:
                for j in range(4):
                    vps[j] = psv.tile([128, 512], f32, tag=f"v{j}",
                                      name=f"v{ch}_{sbh}_{j}")
            for k in ks:
                if load:
                    wt = winp.tile([128, 512], mmdt, tag="win",
                                   name=f"wv{ch}_{k}_{sbh}")
                    nc.gpsimd.dma_start(
                        wt[:], wv[k * 128:(k + 1) * 128,
                                  ch * 512:(ch + 1) * 512])
                    wvtile[k] = wt
                for j in range(4):
                    sb = sbh * 4 + j
                    nc.tensor.matmul(
                        vps[j][:],
                        lhsT=vres[:, k, sb * 128:(sb + 1) * 128],
                        rhs=wvtile[k][:],
                        start=(k == 0), stop=False)
            if close:
                for j in range(4):
                    sb = sbh * 4 + j
                    nc.tensor.matmul(
                        vps[j][:], lhsT=ones_t[:],
                        rhs=bv_t[:, ch * 512:(ch + 1) * 512],
                        start=False, stop=True)
                    nc.scalar.activation(
                        V[:, sb, ch * 512:(ch + 1) * 512], vps[j][:], SILU)

        # ---------------- S1: V rounds + attention ih0 --------------------
        KA, KB = list(range(0, 8)), list(range(8, 16))
        vrounds = [(0, 0, True), (1, 0, True), (1, 1, True), (0, 1, True)]
        for ci, (ch, sbh, load) in enumerate(vrounds):
            hh = 2 * ci            # heads whose scores go into this round
            v_chunk(ch, sbh, KA, True, False, load)
            emit_sc(hh, 0, [0, 1])
            emit_sc(hh, 0, [2, 3])
            if ci > 0:
                new_avp(2 * ci - 2, 0)
                emit_av(2 * ci - 2, 0,
                        [0, 1] if FP8_AV else [0, 1, 2, 3],
                        1 if FP8_AV else 3)
                emit_gated(2 * ci - 2, 0)
            v_chunk(ch, sbh, KB, False, True, load)
            emit_sc(hh + 1, 0, [0, 1])
            emit_sc(hh + 1, 0, [2, 3])
            if ci > 0:
                new_avp(2 * ci - 1, 0)
                emit_av(2 * ci - 1, 0,
                        [0, 1] if FP8_AV else [0, 1, 2, 3],
                        1 if FP8_AV else 3)
                emit_gated(2 * ci - 1, 0)
        for h in (6, 7):
            new_avp(h, 0)
            emit_av(h, 0, [0, 1] if FP8_AV else [0, 1, 2, 3],
                    1 if FP8_AV else 3)
            emit_gated(h, 0)

        psv.release()
        winp.release()
        qresp.release()

        # ---------------- S2: attention ih1 + f2 --------------------------
        w2p = tc.alloc_tile_pool(name="w2p", bufs=1)
        psf2 = tc.alloc_tile_pool(name="psf2", bufs=2, space="PSUM")
        stgp = tc.alloc_tile_pool(name="stgp", bufs=3)

        wf2r = wf2.rearrange("(cb p) n -> p cb n", p=128)
        w2t = {}

        def w2_load(n, slot):
            t = w2p.tile([128, NHL, 512], mmdt, tag=f"w2_{slot}",
                         name=f"w2n{n}")
            # two half loads so the queue isn't blocked for a full 2.8us
            nc.sync.dma_start(t[:, 0:4, :],
                              wf2r[:, 0:4, n * 512:(n + 1) * 512])
            nc.sync.dma_start(t[:, 4:8, :],
                              wf2r[:, 4:8, n * 512:(n + 1) * 512])
            w2t[n] = t

        for n in range(4):
            w2_load(n, n)

        fb = [0]
        f2ps = {}

        def emit_f2_half(n, sb, part):
            if part == 0:
                f2ps[(n, sb)] = psf2.tile([128, 512], f32, tag="f2",
                                          name=f"f2_{n}_{sb}")
            ps = f2ps[(n, sb)]
            for cb in range(part * 4, part * 4 + 4):
                nc.tensor.matmul(
                    ps[:],
                    lhsT=gatedT[:, cb, sb * 128:(sb + 1) * 128],
                    rhs=w2t[n][:, cb, :],
                    start=(cb == 0), stop=(cb == NHL - 1))
            if part == 1:
                fb[0] += 1
                st = stgp.tile([128, 512], f32, tag="st",
                               name=f"st{n}_{sb}")
                if fb[0] % 2 == 0:
                    nc.vector.tensor_copy(st[:], ps[:])
                else:
                    nc.scalar.activation(st[:], ps[:],
                                         mybir.ActivationFunctionType.Copy)
                nc.sync.dma_start(
                    out[sb * 128:(sb + 1) * 128,
                        n * 512:(n + 1) * 512], st[:])
                del f2ps[(n, sb)]

        f2a = [(n, sb) for n in range(4) for sb in range(4)]
        f2b = [(n, sb) for n in range(4) for sb in range(4, 8)]
        halves = [(blk, part) for blk in f2a for part in (0, 1)]
        fi = 0
        for h in range(NHL):
            new_avp(h, 1)
            for pair in range(4):
                emit_sc(h, 1, [2 * pair, 2 * pair + 1])
                if fi >= 2:   # first two slots empty: w2[0] still loading
                    (n, sb), part = halves[fi - 2]
                    emit_f2_half(n, sb, part)
                fi += 1
            if FP8_AV:
                emit_av(h, 1, [0, 1], 3)
                emit_av(h, 1, [2, 3], 3)
            else:
                emit_av(h, 1, [0, 1, 2, 3], 7)
                emit_av(h, 1, [4, 5, 6, 7], 7)
            emit_gated(h, 1)
        for (n, sb), part in halves[30:]:
            emit_f2_half(n, sb, part)
        for n, sb in f2b:
            emit_f2_half(n, sb, 0)
            emit_f2_half(n, sb, 1)

        stgp.release()
        psf2.release()
        w2p.release()
        psv_ = None  # psv already released at S1 end
        psav.release()
        pssc.release()
        atabp.release()
        attscr.release()
        attnp.release()
        bigres.release()
        constp.release()

    nc.compile()
    return nc


def _build_legacy(causal: bool, mm_dt=None):
    """Baseline build (used for the non-causal fallback)."""
    mmdt = {"bf16": bf16, "f32r": f32r}[mm_dt or MM_DT]
    nc = bacc.Bacc("TRN2", target_bir_lowering=False, debug=False,
                   num_devices=NCORES)

    def din(name, shape, dt=f32):
        return nc.dram_tensor(name, shape, dt, kind="ExternalInput").ap()

    qT = din("qT", [H, S], mmdt)
    kT = din("kT", [H, S], mmdt)
    vT = din("vT", [H, S], mmdt)
    wq = din("wq", [H, NHL * HD], mmdt)
    wk = din("wk", [H, NHL * HD], mmdt)
    wv = din("wv", [H, NHL * HD], mmdt)
    wu = din("wu", [H, NHL * HD], mmdt)
    wf2 = din("wf2", [NHL * HD, H], mmdt)
    bq = din("bq", [128, NHL])
    bk = din("bk", [128, NHL])
    bu = din("bu", [128, NHL])
    bv = din("bv", [1, NHL * HD], mmdt)
    ones1 = din("ones1", [1, 128], mmdt)
    atab = din("atab", [NHL, 128, 2047], bf16)
    if not causal:
        maskf = din("maskf", [128, NHL, S], bf16)
    out = nc.dram_tensor("out", [S, H], f32, kind="ExternalOutput").ap()

    with tile.TileContext(nc) as tc:
        with (
            tc.tile_pool(name="const", bufs=1) as constp,
            tc.tile_pool(name="gatedp", bufs=1) as gatedp,
        ):
            bq_t = constp.tile([128, NHL], f32, tag="bq")
            bk_t = constp.tile([128, NHL], f32, tag="bk")
            bu_t = constp.tile([128, NHL], f32, tag="bu")
            bv_t = constp.tile([1, NHL * HD], mmdt, tag="bv")
            ones_t = constp.tile([1, 128], mmdt, tag="ones1")

            gatedT = gatedp.tile([128, NHL, S], mmdt, tag="gatedT")
            wf2r = wf2.rearrange("(cb p) n -> p cb n", p=128)

            with tc.tile_pool(name="attres", bufs=1) as attres:
                UT = attres.tile([128, NHL, S], bf16, tag="UT")
                QT = attres.tile([128, NHL, S], mmdt, tag="QT")
                KTt = attres.tile([128, NHL, S], mmdt, tag="KT")
                V = attres.tile([128, NHL, S], mmdt, tag="V")
                at_tiles = [attres.tile([128, 2047], bf16,
                                        tag=f"atab{h}", name=f"atab{h}")
                            for h in range(NHL)]
                if not causal:
                    mask_t = attres.tile([128, NHL, S], bf16, tag="mask")

                with tc.tile_pool(name="inres", bufs=1) as inres:
                    qres = inres.tile([128, KT16, S], mmdt, tag="qres")
                    kres = inres.tile([128, KT16, S], mmdt, tag="kres")
                    vres = inres.tile([128, KT16, S], mmdt, tag="qres",
                                      name="vres")
                    for k in range(KT16):
                        nc.sync.dma_start(qres[:, k, :],
                                          qT[k * 128:(k + 1) * 128, :])
                    nc.sync.dma_start(bu_t[:], bu[:])
                    nc.sync.dma_start(bq_t[:], bq[:])
                    nc.sync.dma_start(bk_t[:], bk[:])
                    nc.sync.dma_start(bv_t[:], bv[:])
                    nc.sync.dma_start(ones_t[:], ones1[:])
                    for k in range(KT16):
                        nc.sync.dma_start(kres[:, k, :],
                                          kT[k * 128:(k + 1) * 128, :])
                    for k in range(KT16):
                        nc.sync.dma_start(vres[:, k, :],
                                          vT[k * 128:(k + 1) * 128, :])
                    for h in range(NHL):
                        nc.sync.dma_start(at_tiles[h][:], atab[h])
                    if not causal:
                        nc.sync.dma_start(mask_t[:], maskf[:])

                    with (
                        tc.tile_pool(name="win", bufs=6 if causal else 4) as winp,
                        tc.tile_pool(name="pps", bufs=1, space="PSUM") as ppsum,
                    ):
                        for wdram, xres, btile, outtile in (
                            (wu, qres, bu_t, UT),
                            (wq, qres, bq_t, QT),
                            (wk, kres, bk_t, KTt),
                        ):
                            for ih in range(2):
                                ps = [ppsum.tile([128, 512], f32, tag=f"pp{h}",
                                                 name=f"pp{h}")
                                      for h in range(NHL)]
                                for k in range(KT16):
                                    wt = winp.tile([128, NHL * HD], mmdt,
                                                   tag="win")
                                    nc.gpsimd.dma_start(
                                        wt[:], wdram[k * 128:(k + 1) * 128, :])
                                    for h in range(NHL):
                                        nc.tensor.matmul(
                                            ps[h][:],
                                            lhsT=wt[:, h * HD:(h + 1) * HD],
                                            rhs=xres[:, k,
                                                     ih * 512:(ih + 1) * 512],
                                            start=(k == 0),
                                            stop=(k == KT16 - 1))
                                for h in range(NHL):
                                    nc.scalar.activation(
                                        outtile[:, h, ih * 512:(ih + 1) * 512],
                                        ps[h][:], SILU, bias=btile[:, h:h + 1])

                        for ch in range(2):
                            ps = [ppsum.tile([128, 512], f32, tag=f"pp{sb}",
                                             name=f"ppv{sb}")
                                  for sb in range(8)]
                            for k in range(KT16):
                                wt = winp.tile([128, 512], mmdt, tag="wvin")
                                nc.gpsimd.dma_start(
                                    wt[:], wv[k * 128:(k + 1) * 128,
                                              ch * 512:(ch + 1) * 512])
                                for sb in range(8):
                                    nc.tensor.matmul(
                                        ps[sb][:],
                                        lhsT=vres[:, k, sb * 128:(sb + 1) * 128],
                                        rhs=wt[:],
                                        start=(k == 0), stop=False)
                            for sb in range(8):
                                nc.tensor.matmul(
                                    ps[sb][:],
                                    lhsT=ones_t[:],
                                    rhs=bv_t[:, ch * 512:(ch + 1) * 512],
                                    start=False, stop=True)
                                nc.scalar.activation(
                                    V[:, sb, ch * 512:(ch + 1) * 512],
                                    ps[sb][:], SILU)

                with (
                    tc.tile_pool(name="attnp", bufs=4) as attnp,
                    tc.tile_pool(name="psav", bufs=2, space="PSUM") as psav,
                    tc.tile_pool(name="pssc", bufs=4, space="PSUM") as pssc,
                    tc.tile_pool(name="psf2", bufs=2, space="PSUM") as psf2,
                    tc.tile_pool(name="w2p", bufs=8) as w2p,
                    tc.tile_pool(name="stgp", bufs=3) as stgp,
                ):
                    def emit_attention(h, ih):
                        njb = (4 * ih + 4) if causal else 8
                        at = at_tiles[h]
                        avp = psav.tile([128, 512], f32, tag="av",
                                        name=f"av{h}_{ih}")
                        chunks = [list(range(j, min(j + 2, njb)))
                                  for j in range(0, njb, 2)]
                        att_tiles = {}

                        def emit_scores(ch_):
                            for jb in ch_:
                                scp = pssc.tile([128, 512], f32, tag="sc",
                                                name=f"sc{h}_{ih}_{jb}")
                                nc.tensor.matmul(
                                    scp[:],
                                    lhsT=KTt[:, h, jb * 128:(jb + 1) * 128],
                                    rhs=QT[:, h, ih * 512:(ih + 1) * 512],
                                    start=True, stop=True)
                                att = attnp.tile([128, 512], mmdt, tag="attn",
                                                 name=f"at{h}_{ih}_{jb}")
                                d0 = ih * 512 - jb * 128 + MAXLEN - 1
                                nc.vector.scalar_tensor_tensor(
                                    att[:], scp[:], SCALE, at[:, d0:d0 + 512],
                                    op0=MULT, op1=ADD)
                                nc.scalar.activation(att[:], att[:], SILU)
                                if not causal:
                                    nc.vector.tensor_mul(
                                        att[:], att[:],
                                        mask_t[:, jb, ih * 512:(ih + 1) * 512])
                                att_tiles[jb] = att

                        emit_scores(chunks[0])
                        for ci, ch_ in enumerate(chunks):
                            if ci + 1 < len(chunks):
                                emit_scores(chunks[ci + 1])
                            for jb in ch_:
                                nc.tensor.matmul(
                                    avp[:],
                                    lhsT=V[:, jb, h * HD:(h + 1) * HD],
                                    rhs=att_tiles.pop(jb)[:],
                                    start=(jb == 0), stop=(jb == njb - 1))
                        nc.vector.tensor_mul(
                            gatedT[:, h, ih * 512:(ih + 1) * 512],
                            avp[:],
                            UT[:, h, ih * 512:(ih + 1) * 512])

                    def emit_f2_block(w2t, n, sb):
                        ps = psf2.tile([128, 512], f32, tag="f2",
                                       name=f"f2_{n}_{sb}")
                        for cb in range(NHL):
                            nc.tensor.matmul(
                                ps[:],
                                lhsT=gatedT[:, cb, sb * 128:(sb + 1) * 128],
                                rhs=w2t[:, cb, :],
                                start=(cb == 0), stop=(cb == NHL - 1))
                        st = stgp.tile([128, 512], f32, tag="st",
                                       name=f"st{n}_{sb}")
                        nc.vector.tensor_copy(st[:], ps[:])
                        nc.sync.dma_start(
                            out[sb * 128:(sb + 1) * 128,
                                n * 512:(n + 1) * 512], st[:])

                    for h in range(NHL):
                        emit_attention(h, 0)

                    w2a = []
                    for n in range(4):
                        t = w2p.tile([128, NHL, 512], mmdt, tag="w2",
                                     name=f"w2a{n}")
                        nc.sync.dma_start(t[:],
                                          wf2r[:, :, n * 512:(n + 1) * 512])
                        w2a.append(t)

                    fa = [(n, sb) for n in range(4) for sb in range(4)]
                    w2b = []
                    for i in range(NHL):
                        emit_attention(i, 1)
                        for n, sb in fa[2 * i:2 * (i + 1)]:
                            emit_f2_block(w2a[n], n, sb)
                        if i % 2 == 1:
                            t = w2p.tile([128, NHL, 512], mmdt, tag="w2",
                                         name=f"w2b{i // 2}")
                            nc.gpsimd.dma_start(
                                t[:], wf2r[:, :, (i // 2) * 512:
                                           (i // 2 + 1) * 512])
                            w2b.append(t)

                    for n in range(4):
                        for sb in range(4, 8):
                            emit_f2_block(w2b[n], n, sb)

    nc.compile()
    return nc


def _host_shards(query, key, value, attn_mask, Wq, bq, Wk, bk, Wv, bv,
                 Wu, bu, Wf2, rel_table, causal, mm_dt=None):
    """Build the per-core input maps."""
    import ml_dtypes
    npdt = (np.dtype(ml_dtypes.bfloat16) if (mm_dt or MM_DT) == "bf16"
            else np.float32)
    _ONES128 = np.ones((1, 128)).astype(npdt)
    in_maps = []
    # precompute per-head-group weight slices once (shared by 4 cores each)
    gdata = []
    for g in range(HGRP):
        c0, c1 = g * NHL * HD, (g + 1) * NHL * HD
        wq_c = np.ascontiguousarray(Wq[:, c0:c1]).astype(npdt)
        wk_c = np.ascontiguousarray(Wk[:, c0:c1]).astype(npdt)
        wv_c = np.ascontiguousarray(Wv[:, c0:c1]).astype(npdt)
        wu_c = np.ascontiguousarray(Wu[:, c0:c1]).astype(npdt)
        wf2_c = np.ascontiguousarray(Wf2[c0:c1, :]).astype(npdt)
        bq_c = np.ascontiguousarray(bq[c0:c1].reshape(NHL, 128).T)
        bk_c = np.ascontiguousarray(bk[c0:c1].reshape(NHL, 128).T)
        bu_c = np.ascontiguousarray(bu[c0:c1].reshape(NHL, 128).T)
        bv_c = np.ascontiguousarray(bv[c0:c1][None, :]).astype(npdt)
        # atab[h, r, y] = table[y - r, g*NHL + h]; for the causal variant the
        # table is pre-divided by SCALE and masked entries (m < MAXLEN-1,
        # i.e. key index > query index) are -1e5 so silu gives exactly 0.
        y = np.arange(2047)[None, :]
        r = np.arange(128)[:, None]
        idx = y - r                      # [128, 2047]
        valid = (idx >= 0) & (idx <= 2 * MAXLEN - 2)
        idxc = np.clip(idx, 0, 2 * MAXLEN - 2)
        cols = rel_table[:, g * NHL:(g + 1) * NHL]   # [2047, NHL]
        import ml_dtypes as _mld
        if causal:
            cols = np.where(np.arange(2047)[:, None] >= MAXLEN - 1, cols,
                            np.float32(-1e5))
            at = np.where(valid[:, :, None], cols[idxc], np.float32(-1e5))
        else:
            at = cols[idxc] * valid[:, :, None]
        atab_c = np.ascontiguousarray(
            at.transpose(2, 0, 1)).astype(_mld.bfloat16)
        gdata.append((wq_c, wk_c, wv_c, wu_c, wf2_c, bq_c, bk_c, bu_c,
                      bv_c, atab_c))

    for c in range(NCORES):
        b, g = c // HGRP, c % HGRP
        (wq_c, wk_c, wv_c, wu_c, wf2_c, bq_c, bk_c, bu_c, bv_c,
         atab_c) = gdata[g]
        m = {
            "qT": np.ascontiguousarray(query[b].T).astype(npdt),
            "kT": np.ascontiguousarray(key[b].T).astype(npdt),
            "vT": np.ascontiguousarray(value[b].T).astype(npdt),
            "wq": wq_c, "wk": wk_c, "wv": wv_c, "wu": wu_c, "wf2": wf2_c,
            "bq": bq_c, "bk": bk_c, "bu": bu_c, "bv": bv_c, "atab": atab_c,
            "ones1": _ONES128,
        }
        mb = attn_mask[b]
        if not causal:
            import ml_dtypes as _mld
            mf = np.empty((128, NHL, S), _mld.bfloat16)
            for jb in range(8):
                mf[:, jb, :] = mb[:, jb * 128:(jb + 1) * 128].T
            m["maskf"] = mf
        in_maps.append(m)
    return in_maps


def kernel(query, key, value, attn_mask, Wq, bq, Wk, bk, Wv, bv, Wu, bu,
           Wf2, bf2, rel_table):
    global LAST_EXEC_NS, LAST_RES
    query = np.asarray(query, np.float32)
    key = np.asarray(key, np.float32)
    value = np.asarray(value, np.float32)
    attn_mask = np.asarray(attn_mask, bool)
    Wq, bq = np.asarray(Wq, np.float32), np.asarray(bq, np.float32)
    Wk, bk = np.asarray(Wk, np.float32), np.asarray(bk, np.float32)
    Wv, bv = np.asarray(Wv, np.float32), np.asarray(bv, np.float32)
    Wu, bu = np.asarray(Wu, np.float32), np.asarray(bu, np.float32)
    Wf2, bf2 = np.asarray(Wf2, np.float32), np.asarray(bf2, np.float32)
    rel_table = np.asarray(rel_table, np.float32)

    tril = np.tril(np.ones((S, S), bool))
    causal = all(np.array_equal(attn_mask[b], tril) for b in range(B))

    key_ = (causal, MM_DT, "v2")
    if key_ not in _CACHE:
        _CACHE[key_] = (_build_v2() if causal
                        else _build_legacy(causal))
    nc = _CACHE[key_]

    in_maps = _host_shards(query, key, value, attn_mask, Wq, bq, Wk, bk,
                           Wv, bv, Wu, bu, Wf2, rel_table, causal)
    res = run_bass_kernel_spmd(nc, in_maps, list(range(NCORES)), trace=TRACE)
    LAST_RES = res
    if res.exec_time_ns is not None:
        LAST_EXEC_NS = res.exec_time_ns

    outp = np.empty((B, S, H), np.float32)
    for b in range(B):
        outp[b] = (res.results[2 * b]["out"] + res.results[2 * b + 1]["out"]
                   + bf2[None, :])
    return outp


# revision 19
# speedup vs baseline: 1.0134x; 1.0134x over previous
"""Trainium2 Bass kernel for nn_BaselineModel_35175782154746 (dense transformer
block with SiLU attention + relative-position bias).

Sharding: 8 NeuronCores = 4 batches x 2 head-groups (8 heads each).
Each core computes, for its (batch b, head-group g):
    U, Q, K, V projections (columns g*1024:(g+1)*1024 of Wu/Wq/Wk/Wv),
    SiLU attention with rel-pos bias for its 8 heads,
    gated = out * U, partial = gated @ Wf2[g*1024:(g+1)*1024, :].
Host reduces: out[b] = partial[2b] + partial[2b+1] + bf2.

All matmuls run with bf16 operands (fp32 PSUM accumulation). The causal
build exploits causality exactly: score / bias / silu / AV work is trimmed
to queries >= key-block start (the above-diagonal wedge inside the
diagonal 128-block is masked via the -1e5 entries baked into the shifted
bias table, which silu maps to an exact 0).

Scheduling (causal build): projections run in 4-head PSUM rounds
alternating two 4-bank groups so activation drain overlaps the next
round's matmuls (the last round uses the group whose banks the V rounds
do NOT reuse); the V projection, attention ih=0 and f2 are emitted as
one interleaved stream so the TensorEngine never idles (idle gaps also
drop it out of its max p-state). PSUM-reading elementwise ops live on
Vector (GpSimd cannot access PSUM); half the f2 copies go through the
Scalar engine's Copy activation. fp8 DoubleRow variants of the score /
AV matmuls exist behind FP8_SC / FP8_AV but are disabled: measured
end-to-end they were slower (pair-granular causal trim outweighs the
2x rate) and cost ~1.1e-2 extra relative error.
"""

import sys
import os

for _p in ("/root/.axon_site/_ro/trn_rl_repo", "/opt/trn_rl_repo"):
    if os.path.isdir(_p) and _p not in sys.path:
        sys.path.append(_p)

import numpy as np

import concourse.bass as bass
import concourse.mybir as mybir
import concourse.tile as tile
from concourse import bacc
from concourse.bass_utils import run_bass_kernel_spmd

B, S, H, NH, MAXLEN = 4, 1024, 2048, 16, 1024
HD = H // NH            # 128
NHL = 8                 # heads per core (local)
HGRP = 2                # head groups
NCORES = 8
KT16 = H // 128         # 16 k-tiles for the H contraction
SCALE = float(HD) ** -0.5

f32 = mybir.dt.float32
f32r = mybir.dt.float32r
bf16 = mybir.dt.bfloat16
f8 = mybir.dt.float8e4
SILU = mybir.ActivationFunctionType.Silu
MULT = mybir.AluOpType.mult
ADD = mybir.AluOpType.add
DR = mybir.MatmulPerfMode.DoubleRow
FP8_SC = False          # fp8 DoubleRow score matmuls (Q/K repacked [64,2,.])
FP8_AV = False          # fp8 DoubleRow AV matmuls (V + attention probs fp8)

TRACE = False
LAST_EXEC_NS = None
LAST_RES = None
MM_DT = "bf16"          # "bf16" or "f32r" matmul operand dtype
_CACHE = {}


def _build_v2(mm_dt=None):
    """Causal-only build with interleaved emission."""
    mmdt = {"bf16": bf16, "f32r": f32r}[mm_dt or MM_DT]
    nc = bacc.Bacc("TRN2", target_bir_lowering=False, debug=False,
                   num_devices=NCORES)

    def din(name, shape, dt=f32):
        return nc.dram_tensor(name, shape, dt, kind="ExternalInput").ap()

    qT = din("qT", [H, S], mmdt)
    kT = din("kT", [H, S], mmdt)
    vT = din("vT", [H, S], mmdt)
    wq = din("wq", [H, NHL * HD], mmdt)
    wk = din("wk", [H, NHL * HD], mmdt)
    wv = din("wv", [H, NHL * HD], mmdt)
    wu = din("wu", [H, NHL * HD], mmdt)
    wf2 = din("wf2", [NHL * HD, H], mmdt)
    bq = din("bq", [128, NHL])
    bk = din("bk", [128, NHL])
    bu = din("bu", [128, NHL])
    bv = din("bv", [1, NHL * HD], mmdt)
    ones1 = din("ones1", [1, 128], mmdt)
    atab = din("atab", [NHL, 128, 2047], bf16)
    out = nc.dram_tensor("out", [S, H], f32, kind="ExternalOutput").ap()

    with tile.TileContext(nc) as tc:
        constp = tc.alloc_tile_pool(name="const", bufs=1)
        bigres = tc.alloc_tile_pool(name="bigres", bufs=1)
        attnp = tc.alloc_tile_pool(name="attnp", bufs=6 if FP8_AV else 12)
        attscr = tc.alloc_tile_pool(name="attscr", bufs=4)
        atabp = tc.alloc_tile_pool(name="atabp", bufs=1)
        qresp = tc.alloc_tile_pool(name="qresp", bufs=1)
        winp = tc.alloc_tile_pool(name="winp", bufs=8)
        kresp = tc.alloc_tile_pool(name="kresp", bufs=1)

        bq_t = constp.tile([128, NHL], f32, tag="bq")
        bk_t = constp.tile([128, NHL], f32, tag="bk")
        bu_t = constp.tile([128, NHL], f32, tag="bu")
        bv_t = constp.tile([1, NHL * HD], mmdt, tag="bv")
        ones_t = constp.tile([1, 128], mmdt, tag="ones1")

        qk_dt = f8 if FP8_SC else mmdt
        av_dt = f8 if FP8_AV else mmdt
        UT = bigres.tile([128, NHL, S], bf16, tag="UT")
        QT = bigres.tile([128, NHL, S], qk_dt, tag="QT")
        KTt = bigres.tile([128, NHL, S], qk_dt, tag="KT")
        V = bigres.tile([128, NHL, S], av_dt, tag="V")
        gatedT = bigres.tile([128, NHL, S], mmdt, tag="gatedT")
        if FP8_SC:
            # [64, 2, h, s]: head-dim split into 2 k-tiles of 64 partitions
            # for the DoubleRow score matmul; filled by SBUF-SBUF repack.
            Q8 = bigres.tile([64, 2, NHL, S], f8, tag="Q8")
            K8 = bigres.tile([64, 2, NHL, S], f8, tag="K8")

        qres = qresp.tile([128, KT16, S], mmdt, tag="qres")
        kres = kresp.tile([128, KT16, S], mmdt, tag="kres")
        # vres shares qres's slot: qres's last read is the Q phase and the
        # vres load lands during K.
        vres = qresp.tile([128, KT16, S], mmdt, tag="qres", name="vres")

        # ---- input DMAs: first q k-tile first so U can start ASAP ----
        nc.sync.dma_start(qres[:, 0, :], qT[0:128, :])
        nc.sync.dma_start(bu_t[:], bu[:])
        nc.sync.dma_start(bq_t[:], bq[:])
        nc.sync.dma_start(bk_t[:], bk[:])
        nc.sync.dma_start(bv_t[:], bv[:])
        nc.sync.dma_start(ones_t[:], ones1[:])
        for k in range(1, KT16):
            nc.sync.dma_start(qres[:, k, :], qT[k * 128:(k + 1) * 128, :])
        for k in range(KT16):
            nc.sync.dma_start(kres[:, k, :], kT[k * 128:(k + 1) * 128, :])
        for k in range(KT16):
            nc.sync.dma_start(vres[:, k, :], vT[k * 128:(k + 1) * 128, :])
        at_tiles = [atabp.tile([128, 2047], bf16, tag=f"atab{h}",
                               name=f"atab{h}")
                    for h in range(NHL)]
        for h in range(NHL):
            nc.sync.dma_start(at_tiles[h][:], atab[h])

        # ================= U, Q, K projections =================
        # 4-head rounds, two alternating 4-bank PSUM groups; weight k-tiles
        # [128, 512] persist across both query halves of a round-pair.
        pproj = tc.alloc_tile_pool(name="pproj", bufs=1, space="PSUM")
        rnd = [0]
        wtile = {}

        def proj_round(wdram, xres, btile, outtile, half, ih):
            grp = ((rnd[0] + 1) % 2) * 4
            rnd[0] += 1
            ps = [pproj.tile([128, 512], f32, tag=f"pp{grp + j}",
                             name=f"pp{rnd[0]}_{j}")
                  for j in range(4)]
            for k in range(KT16):
                wt = winp.tile([128, 512], mmdt, tag="win",
                               name=f"w{k}_{rnd[0]}")
                qeng = nc.scalar if (rnd[0] == 1 and k % 2) else nc.gpsimd
                qeng.dma_start(
                    wt[:], wdram[k * 128:(k + 1) * 128,
                                 half * 512:(half + 1) * 512])
                for j in range(4):
                    nc.tensor.matmul(
                        ps[j][:],
                        lhsT=wt[:, j * HD:(j + 1) * HD],
                        rhs=xres[:, k, ih * 512:(ih + 1) * 512],
                        start=(k == 0), stop=(k == KT16 - 1))
            for j in range(4):
                h = half * 4 + j
                nc.scalar.activation(
                    outtile[:, h, ih * 512:(ih + 1) * 512],
                    ps[j][:], SILU, bias=btile[:, h:h + 1])

        def proj_all(wdram, xres, btile, outtile):
            for half in range(2):
                for ih in range(2):
                    proj_round(wdram, xres, btile, outtile, half, ih)

        proj_all(wu, qres, bu_t, UT)
        proj_all(wq, qres, bq_t, QT)
        if FP8_SC:
            # repack Q to [64, 2, h, s] during the K projection
            nc.scalar.dma_start(Q8[:, 0], QT[0:64])
            nc.scalar.dma_start(Q8[:, 1], QT[64:128])
        proj_all(wk, kres, bk_t, KTt)
        if FP8_SC:
            nc.scalar.dma_start(K8[:, 0], KTt[0:64])
            nc.scalar.dma_start(K8[:, 1], KTt[64:128])

        pproj.release()
        kresp.release()

        # ============ V projection + attention + f2: one stream ===========
        pssc = tc.alloc_tile_pool(name="pssc", bufs=3, space="PSUM")
        psav = tc.alloc_tile_pool(name="psav", bufs=1, space="PSUM")
        psv = tc.alloc_tile_pool(name="psv", bufs=1, space="PSUM")

        scn = [0]
        attq = {}
        avps = {}

        def emit_sc(h, ih, jbs):
            """Score + bias + silu for (h, ih, jb in jbs), trimmed to the
            causal query range. With FP8_AV the trim is pair-granular (the
            odd block's extra wedge is silu(-1e5) = 0, needed since the AV
            matmul consumes both pair planes over the same column range)."""
            at = at_tiles[h]
            for jb in jbs:
                if FP8_AV:
                    q0 = max(0, (jb // 2) * 256 - ih * 512)
                else:
                    q0 = max(0, jb * 128 - ih * 512)
                scp = pssc.tile([128, 512], f32, tag="sc",
                                name=f"sc{h}_{ih}_{jb}")
                if FP8_SC:
                    nc.tensor.matmul(
                        scp[:, q0:512],
                        lhsT=K8[:, :, h, jb * 128:(jb + 1) * 128],
                        rhs=Q8[:, :, h, ih * 512 + q0:(ih + 1) * 512],
                        start=True, stop=True, perf_mode=DR)
                else:
                    nc.tensor.matmul(
                        scp[:, q0:512],
                        lhsT=KTt[:, h, jb * 128:(jb + 1) * 128],
                        rhs=QT[:, h, ih * 512 + q0:(ih + 1) * 512],
                        start=True, stop=True)
                d0 = ih * 512 - jb * 128 + MAXLEN - 1
                if FP8_AV:
                    if jb % 2 == 0:
                        attq[(h, ih, jb // 2)] = attnp.tile(
                            [128, 2, 512], f8, tag="att",
                            name=f"at{h}_{ih}_{jb // 2}")
                    pair = attq[(h, ih, jb // 2)]
                    scr = attscr.tile([128, 512], bf16, tag="scr",
                                      name=f"scr{h}_{ih}_{jb}")
                    nc.vector.scalar_tensor_tensor(
                        scr[:, q0:512], scp[:, q0:512], SCALE,
                        at[:, d0 + q0:d0 + 512], op0=MULT, op1=ADD)
                    nc.scalar.activation(pair[:, jb % 2, q0:512],
                                         scr[:, q0:512], SILU)
                else:
                    att = attnp.tile([128, 512], mmdt, tag="att",
                                     name=f"at{h}_{ih}_{jb}")
                    nc.vector.scalar_tensor_tensor(
                        att[:, q0:512], scp[:, q0:512], SCALE,
                        at[:, d0 + q0:d0 + 512], op0=MULT, op1=ADD)
                    nc.scalar.activation(att[:, q0:512], att[:, q0:512],
                                         SILU)
                    attq[(h, ih, jb)] = att

        def emit_av(h, ih, jbs, last_jb):
            avp = avps[(h, ih)]
            if FP8_AV:
                for p in jbs:
                    q0 = max(0, p * 256 - ih * 512)
                    nc.tensor.matmul(
                        avp[:, q0:512],
                        lhsT=V[:, 2 * p:2 * p + 2, h * HD:(h + 1) * HD],
                        rhs=attq.pop((h, ih, p))[:, :, q0:512],
                        start=(p == 0), stop=(p == last_jb),
                        perf_mode=DR, skip_group_check=True)
            else:
                for jb in jbs:
                    q0 = max(0, jb * 128 - ih * 512)
                    nc.tensor.matmul(
                        avp[:, q0:512],
                        lhsT=V[:, jb, h * HD:(h + 1) * HD],
                        rhs=attq.pop((h, ih, jb))[:, q0:512],
                        start=(jb == 0), stop=(jb == last_jb),
                        skip_group_check=True)

        def new_avp(h, ih):
            avps[(h, ih)] = psav.tile([128, 512], f32, tag="av",
                                      name=f"av{h}_{ih}")

        def emit_gated(h, ih):
            nc.vector.scalar_tensor_tensor(
                gatedT[:, h, ih * 512:(ih + 1) * 512],
                avps.pop((h, ih))[:], 1.0,
                UT[:, h, ih * 512:(ih + 1) * 512],
                op0=MULT, op1=MULT)

        # V rounds: (ch, sbh) in order (0,0) (1,0) (1,1) (0,1) so that AV
        # of ih=0 (key blocks 0-3 = sbh 0) unlocks after two rounds and
        # the ch1 weight tiles are reused across adjacent rounds. wv tiles
        # reuse the projection weight tags (their last reads precede V).
        vps = {}
        wvtile = {}

        def v_chunk(ch, sbh, ks, open_, close, load):
            if open_:
                for j in range(4):
                    vps[j] = psv.tile([128, 512], f32, tag=f"v{j}",
                                      name=f"v{ch}_{sbh}_{j}")
            for k in ks:
                if load:
                    wt = winp.tile([128, 512], mmdt, tag="win",
                                   name=f"wv{ch}_{k}_{sbh}")
                    nc.gpsimd.dma_start(
                        wt[:], wv[k * 128:(k + 1) * 128,
                                  ch * 512:(ch + 1) * 512])
                    wvtile[k] = wt
                for j in range(4):
                    sb = sbh * 4 + j
                    nc.tensor.matmul(
                        vps[j][:],
                        lhsT=vres[:, k, sb * 128:(sb + 1) * 128],
                        rhs=wvtile[k][:],
                        start=(k == 0), stop=False)
            if close:
                for j in range(4):
                    sb = sbh * 4 + j
                    nc.tensor.matmul(
                        vps[j][:], lhsT=ones_t[:],
                        rhs=bv_t[:, ch * 512:(ch + 1) * 512],
                        start=False, stop=True)
                    nc.scalar.activation(
                        V[:, sb, ch * 512:(ch + 1) * 512], vps[j][:], SILU)

        # ---------------- S1: V rounds + attention ih0 --------------------
        KA, KB = list(range(0, 8)), list(range(8, 16))
        vrounds = [(0, 0, True), (1, 0, True), (1, 1, True), (0, 1, True)]
        for ci, (ch, sbh, load) in enumerate(vrounds):
            hh = 2 * ci            # heads whose scores go into this round
            v_chunk(ch, sbh, KA, True, False, load)
            emit_sc(hh, 0, [0, 1])
            emit_sc(hh, 0, [2, 3])
            if ci > 0:
                new_avp(2 * ci - 2, 0)
                emit_av(2 * ci - 2, 0,
                        [0, 1] if FP8_AV else [0, 1, 2, 3],
                        1 if FP8_AV else 3)
                emit_gated(2 * ci - 2, 0)
            v_chunk(ch, sbh, KB, False, True, load)
            emit_sc(hh + 1, 0, [0, 1])
            emit_sc(hh + 1, 0, [2, 3])
            if ci > 0:
                new_avp(2 * ci - 1, 0)
                emit_av(2 * ci - 1, 0,
                        [0, 1] if FP8_AV else [0, 1, 2, 3],
                        1 if FP8_AV else 3)
                emit_gated(2 * ci - 1, 0)
        for h in (6, 7):
            new_avp(h, 0)
            emit_av(h, 0, [0, 1] if FP8_AV else [0, 1, 2, 3],
                    1 if FP8_AV else 3)
            emit_gated(h, 0)

        psv.release()
        winp.release()
        qresp.release()

        # ---------------- S2: attention ih1 + f2 --------------------------
        w2p = tc.alloc_tile_pool(name="w2p", bufs=1)
        psf2 = tc.alloc_tile_pool(name="psf2", bufs=2, space="PSUM")
        stgp = tc.alloc_tile_pool(name="stgp", bufs=3)

        wf2r = wf2.rearrange("(cb p) n -> p cb n", p=128)
        w2t = {}

        def w2_load(n, slot):
            t = w2p.tile([128, NHL, 512], mmdt, tag=f"w2_{slot}",
                         name=f"w2n{n}")
            # two half loads so the queue isn't blocked for a full 2.8us
            nc.sync.dma_start(t[:, 0:4, :],
                              wf2r[:, 0:4, n * 512:(n + 1) * 512])
            nc.sync.dma_start(t[:, 4:8, :],
                              wf2r[:, 4:8, n * 512:(n + 1) * 512])
            w2t[n] = t

        for n in range(4):
            w2_load(n, n)

        fb = [0]
        f2ps = {}

        def emit_f2_half(n, sb, part):
            if part == 0:
                f2ps[(n, sb)] = psf2.tile([128, 512], f32, tag="f2",
                                          name=f"f2_{n}_{sb}")
            ps = f2ps[(n, sb)]
            for cb in range(part * 4, part * 4 + 4):
                nc.tensor.matmul(
                    ps[:],
                    lhsT=gatedT[:, cb, sb * 128:(sb + 1) * 128],
                    rhs=w2t[n][:, cb, :],
                    start=(cb == 0), stop=(cb == NHL - 1))
            if part == 1:
                fb[0] += 1
                st = stgp.tile([128, 512], f32, tag="st",
                               name=f"st{n}_{sb}")
                if fb[0] % 2 == 0:
                    nc.vector.tensor_copy(st[:], ps[:])
                else:
                    nc.scalar.activation(st[:], ps[:],
                                         mybir.ActivationFunctionType.Copy)
                nc.sync.dma_start(
                    out[sb * 128:(sb + 1) * 128,
                        n * 512:(n + 1) * 512], st[:])
                del f2ps[(n, sb)]

        f2a = [(n, sb) for n in range(4) for sb in range(4)]
        f2b = [(n, sb) for n in range(4) for sb in range(4, 8)]
        halves = [(blk, part) for blk in f2a for part in (0, 1)]
        fi = 0
        for h in range(NHL):
            new_avp(h, 1)
            for pair in range(4):
                emit_sc(h, 1, [2 * pair, 2 * pair + 1])
                if fi >= 2:   # first two slots empty: w2[0] still loading
                    (n, sb), part = halves[fi - 2]
                    emit_f2_half(n, sb, part)
                fi += 1
            if FP8_AV:
                emit_av(h, 1, [0, 1], 3)
                emit_av(h, 1, [2, 3], 3)
            else:
                emit_av(h, 1, [0, 1, 2, 3], 7)
                emit_av(h, 1, [4, 5, 6, 7], 7)
            emit_gated(h, 1)
        for (n, sb), part in halves[30:]:
            emit_f2_half(n, sb, part)
        for n, sb in f2b:
            emit_f2_half(n, sb, 0)
            emit_f2_half(n, sb, 1)

        stgp.release()
        psf2.release()
        w2p.release()
        psv_ = None  # psv already released at S1 end
        psav.release()
        pssc.release()
        atabp.release()
        attscr.release()
        attnp.release()
        bigres.release()
        constp.release()

    nc.compile()
    return nc


def _build_legacy(causal: bool, mm_dt=None):
    """Baseline build (used for the non-causal fallback)."""
    mmdt = {"bf16": bf16, "f32r": f32r}[mm_dt or MM_DT]
    nc = bacc.Bacc("TRN2", target_bir_lowering=False, debug=False,
                   num_devices=NCORES)

    def din(name, shape, dt=f32):
        return nc.dram_tensor(name, shape, dt, kind="ExternalInput").ap()

    qT = din("qT", [H, S], mmdt)
    kT = din("kT", [H, S], mmdt)
    vT = din("vT", [H, S], mmdt)
    wq = din("wq", [H, NHL * HD], mmdt)
    wk = din("wk", [H, NHL * HD], mmdt)
    wv = din("wv", [H, NHL * HD], mmdt)
    wu = din("wu", [H, NHL * HD], mmdt)
    wf2 = din("wf2", [NHL * HD, H], mmdt)
    bq = din("bq", [128, NHL])
    bk = din("bk", [128, NHL])
    bu = din("bu", [128, NHL])
    bv = din("bv", [1, NHL * HD], mmdt)
    ones1 = din("ones1", [1, 128], mmdt)
    atab = din("atab", [NHL, 128, 2047], bf16)
    if not causal:
        maskf = din("maskf", [128, NHL, S], bf16)
    out = nc.dram_tensor("out", [S, H], f32, kind="ExternalOutput").ap()

    with tile.TileContext(nc) as tc:
        with (
            tc.tile_pool(name="const", bufs=1) as constp,
            tc.tile_pool(name="gatedp", bufs=1) as gatedp,
        ):
            bq_t = constp.tile([128, NHL], f32, tag="bq")
            bk_t = constp.tile([128, NHL], f32, tag="bk")
            bu_t = constp.tile([128, NHL], f32, tag="bu")
            bv_t = constp.tile([1, NHL * HD], mmdt, tag="bv")
            ones_t = constp.tile([1, 128], mmdt, tag="ones1")

            gatedT = gatedp.tile([128, NHL, S], mmdt, tag="gatedT")
            wf2r = wf2.rearrange("(cb p) n -> p cb n", p=128)

            with tc.tile_pool(name="attres", bufs=1) as attres:
                UT = attres.tile([128, NHL, S], bf16, tag="UT")
                QT = attres.tile([128, NHL, S], mmdt, tag="QT")
                KTt = attres.tile([128, NHL, S], mmdt, tag="KT")
                V = attres.tile([128, NHL, S], mmdt, tag="V")
                at_tiles = [attres.tile([128, 2047], bf16,
                                        tag=f"atab{h}", name=f"atab{h}")
                            for h in range(NHL)]
                if not causal:
                    mask_t = attres.tile([128, NHL, S], bf16, tag="mask")

                with tc.tile_pool(name="inres", bufs=1) as inres:
                    qres = inres.tile([128, KT16, S], mmdt, tag="qres")
                    kres = inres.tile([128, KT16, S], mmdt, tag="kres")
                    vres = inres.tile([128, KT16, S], mmdt, tag="qres",
                                      name="vres")
                    for k in range(KT16):
                        nc.sync.dma_start(qres[:, k, :],
                                          qT[k * 128:(k + 1) * 128, :])
                    nc.sync.dma_start(bu_t[:], bu[:])
                    nc.sync.dma_start(bq_t[:], bq[:])
                    nc.sync.dma_start(bk_t[:], bk[:])
                    nc.sync.dma_start(bv_t[:], bv[:])
                    nc.sync.dma_start(ones_t[:], ones1[:])
                    for k in range(KT16):
                        nc.sync.dma_start(kres[:, k, :],
                                          kT[k * 128:(k + 1) * 128, :])
                    for k in range(KT16):
                        nc.sync.dma_start(vres[:, k, :],
                                          vT[k * 128:(k + 1) * 128, :])
                    for h in range(NHL):
                        nc.sync.dma_start(at_tiles[h][:], atab[h])
                    if not causal:
                        nc.sync.dma_start(mask_t[:], maskf[:])

                    with (
                        tc.tile_pool(name="win", bufs=6 if causal else 4) as winp,
                        tc.tile_pool(name="pps", bufs=1, space="PSUM") as ppsum,
                    ):
                        for wdram, xres, btile, outtile in (
                            (wu, qres, bu_t, UT),
                            (wq, qres, bq_t, QT),
                            (wk, kres, bk_t, KTt),
                        ):
                            for ih in range(2):
                                ps = [ppsum.tile([128, 512], f32, tag=f"pp{h}",
                                                 name=f"pp{h}")
                                      for h in range(NHL)]
                                for k in range(KT16):
                                    wt = winp.tile([128, NHL * HD], mmdt,
                                                   tag="win")
                                    nc.gpsimd.dma_start(
                                        wt[:], wdram[k * 128:(k + 1) * 128, :])
                                    for h in range(NHL):
                                        nc.tensor.matmul(
                                            ps[h][:],
                                            lhsT=wt[:, h * HD:(h + 1) * HD],
                                            rhs=xres[:, k,
                                                     ih * 512:(ih + 1) * 512],
                                            start=(k == 0),
                                            stop=(k == KT16 - 1))
                                for h in range(NHL):
                                    nc.scalar.activation(
                                        outtile[:, h, ih * 512:(ih + 1) * 512],
                                        ps[h][:], SILU, bias=btile[:, h:h + 1])

                        for ch in range(2):
                            ps = [ppsum.tile([128, 512], f32, tag=f"pp{sb}",
                                             name=f"ppv{sb}")
                                  for sb in range(8)]
                            for k in range(KT16):
                                wt = winp.tile([128, 512], mmdt, tag="wvin")
                                nc.gpsimd.dma_start(
                                    wt[:], wv[k * 128:(k + 1) * 128,
                                              ch * 512:(ch + 1) * 512])
                                for sb in range(8):
                                    nc.tensor.matmul(
                                        ps[sb][:],
                                        lhsT=vres[:, k, sb * 128:(sb + 1) * 128],
                                        rhs=wt[:],
                                        start=(k == 0), stop=False)
                            for sb in range(8):
                                nc.tensor.matmul(
                                    ps[sb][:],
                                    lhsT=ones_t[:],
                                    rhs=bv_t[:, ch * 512:(ch + 1) * 512],
                                    start=False, stop=True)
                                nc.scalar.activation(
                                    V[:, sb, ch * 512:(ch + 1) * 512],
                                    ps[sb][:], SILU)

                with (
                    tc.tile_pool(name="attnp", bufs=4) as attnp,
                    tc.tile_pool(name="psav", bufs=2, space="PSUM") as psav,
                    tc.tile_pool(name="pssc", bufs=4, space="PSUM") as pssc,
                    tc.tile_pool(name="psf2", bufs=2, space="PSUM") as psf2,
                    tc.tile_pool(name="w2p", bufs=8) as w2p,
                    tc.tile_pool(name="stgp", bufs=3) as stgp,
                ):
                    def emit_attention(h, ih):
                        njb = (4 * ih + 4) if causal else 8
                        at = at_tiles[h]
                        avp = psav.tile([128, 512], f32, tag="av",
                                        name=f"av{h}_{ih}")
                        chunks = [list(range(j, min(j + 2, njb)))
                                  for j in range(0, njb, 2)]
                        att_tiles = {}

                        def emit_scores(ch_):
                            for jb in ch_:
                                scp = pssc.tile([128, 512], f32, tag="sc",
                                                name=f"sc{h}_{ih}_{jb}")
                                nc.tensor.matmul(
                                    scp[:],
                                    lhsT=KTt[:, h, jb * 128:(jb + 1) * 128],
                                    rhs=QT[:, h, ih * 512:(ih + 1) * 512],
                                    start=True, stop=True)
                                att = attnp.tile([128, 512], mmdt, tag="attn",
                                                 name=f"at{h}_{ih}_{jb}")
                                d0 = ih * 512 - jb * 128 + MAXLEN - 1
                                nc.vector.scalar_tensor_tensor(
                                    att[:], scp[:], SCALE, at[:, d0:d0 + 512],
                                    op0=MULT, op1=ADD)
                                nc.scalar.activation(att[:], att[:], SILU)
                                if not causal:
                                    nc.vector.tensor_mul(
                                        att[:], att[:],
                                        mask_t[:, jb, ih * 512:(ih + 1) * 512])
                                att_tiles[jb] = att

                        emit_scores(chunks[0])
                        for ci, ch_ in enumerate(chunks):
                            if ci + 1 < len(chunks):
                                emit_scores(chunks[ci + 1])
                            for jb in ch_:
                                nc.tensor.matmul(
                                    avp[:],
                                    lhsT=V[:, jb, h * HD:(h + 1) * HD],
                                    rhs=att_tiles.pop(jb)[:],
                                    start=(jb == 0), stop=(jb == njb - 1))
                        nc.vector.tensor_mul(
                            gatedT[:, h, ih * 512:(ih + 1) * 512],
                            avp[:],
                            UT[:, h, ih * 512:(ih + 1) * 512])

                    def emit_f2_block(w2t, n, sb):
                        ps = psf2.tile([128, 512], f32, tag="f2",
                                       name=f"f2_{n}_{sb}")
                        for cb in range(NHL):
                            nc.tensor.matmul(
                                ps[:],
                                lhsT=gatedT[:, cb, sb * 128:(sb + 1) * 128],
                                rhs=w2t[:, cb, :],
                                start=(cb == 0), stop=(cb == NHL - 1))
                        st = stgp.tile([128, 512], f32, tag="st",
                                       name=f"st{n}_{sb}")
                        nc.vector.tensor_copy(st[:], ps[:])
                        nc.sync.dma_start(
                            out[sb * 128:(sb + 1) * 128,
                                n * 512:(n + 1) * 512], st[:])

                    for h in range(NHL):
                        emit_attention(h, 0)

                    w2a = []
                    for n in range(4):
                        t = w2p.tile([128, NHL, 512], mmdt, tag="w2",
                                     name=f"w2a{n}")
                        nc.sync.dma_start(t[:],
                                          wf2r[:, :, n * 512:(n + 1) * 512])
                        w2a.append(t)

                    fa = [(n, sb) for n in range(4) for sb in range(4)]
                    w2b = []
                    for i in range(NHL):
                        emit_attention(i, 1)
                        for n, sb in fa[2 * i:2 * (i + 1)]:
                            emit_f2_block(w2a[n], n, sb)
                        if i % 2 == 1:
                            t = w2p.tile([128, NHL, 512], mmdt, tag="w2",
                                         name=f"w2b{i // 2}")
                            nc.gpsimd.dma_start(
                                t[:], wf2r[:, :, (i // 2) * 512:
                                           (i // 2 + 1) * 512])
                            w2b.append(t)

                    for n in range(4):
                        for sb in range(4, 8):
                            emit_f2_block(w2b[n], n, sb)

    nc.compile()
    return nc


def _host_shards(query, key, value, attn_mask, Wq, bq, Wk, bk, Wv, bv,
                 Wu, bu, Wf2, rel_table, causal, mm_dt=None):
    """Build the per-core input maps."""
    import ml_dtypes
    npdt = (np.dtype(ml_dtypes.bfloat16) if (mm_dt or MM_DT) == "bf16"
            else np.float32)
    _ONES128 = np.ones((1, 128)).astype(npdt)
    in_maps = []
    # precompute per-head-group weight slices once (shared by 4 cores each)
    gdata = []
    for g in range(HGRP):
        c0, c1 = g * NHL * HD, (g + 1) * NHL * HD
        wq_c = np.ascontiguousarray(Wq[:, c0:c1]).astype(npdt)
        wk_c = np.ascontiguousarray(Wk[:, c0:c1]).astype(npdt)
        wv_c = np.ascontiguousarray(Wv[:, c0:c1]).astype(npdt)
        wu_c = np.ascontiguousarray(Wu[:, c0:c1]).astype(npdt)
        wf2_c = np.ascontiguousarray(Wf2[c0:c1, :]).astype(npdt)
        bq_c = np.ascontiguousarray(bq[c0:c1].reshape(NHL, 128).T)
        bk_c = np.ascontiguousarray(bk[c0:c1].reshape(NHL, 128).T)
        bu_c = np.ascontiguousarray(bu[c0:c1].reshape(NHL, 128).T)
        bv_c = np.ascontiguousarray(bv[c0:c1][None, :]).astype(npdt)
        # atab[h, r, y] = table[y - r, g*NHL + h]; for the causal variant the
        # table is pre-divided by SCALE and masked entries (m < MAXLEN-1,
        # i.e. key index > query index) are -1e5 so silu gives exactly 0.
        y = np.arange(2047)[None, :]
        r = np.arange(128)[:, None]
        idx = y - r                      # [128, 2047]
        valid = (idx >= 0) & (idx <= 2 * MAXLEN - 2)
        idxc = np.clip(idx, 0, 2 * MAXLEN - 2)
        cols = rel_table[:, g * NHL:(g + 1) * NHL]   # [2047, NHL]
        import ml_dtypes as _mld
        if causal:
            cols = np.where(np.arange(2047)[:, None] >= MAXLEN - 1, cols,
                            np.float32(-1e5))
            at = np.where(valid[:, :, None], cols[idxc], np.float32(-1e5))
        else:
            at = cols[idxc] * valid[:, :, None]
        atab_c = np.ascontiguousarray(
            at.transpose(2, 0, 1)).astype(_mld.bfloat16)
        gdata.append((wq_c, wk_c, wv_c, wu_c, wf2_c, bq_c, bk_c, bu_c,
                      bv_c, atab_c))

    for c in range(NCORES):
        b, g = c // HGRP, c % HGRP
        (wq_c, wk_c, wv_c, wu_c, wf2_c, bq_c, bk_c, bu_c, bv_c,
         atab_c) = gdata[g]
        m = {
            "qT": np.ascontiguousarray(query[b].T).astype(npdt),
            "kT": np.ascontiguousarray(key[b].T).astype(npdt),
            "vT": np.ascontiguousarray(value[b].T).astype(npdt),
            "wq": wq_c, "wk": wk_c, "wv": wv_c, "wu": wu_c, "wf2": wf2_c,
            "bq": bq_c, "bk": bk_c, "bu": bu_c, "bv": bv_c, "atab": atab_c,
            "ones1": _ONES128,
        }
        mb = attn_mask[b]
        if not causal:
            import ml_dtypes as _mld
            mf = np.empty((128, NHL, S), _mld.bfloat16)
            for jb in range(8):
                mf[:, jb, :] = mb[:, jb * 128:(jb + 1) * 128].T
            m["maskf"] = mf
        in_maps.append(m)
    return in_maps


def kernel(query, key, value, attn_mask, Wq, bq, Wk, bk, Wv, bv, Wu, bu,
           Wf2, bf2, rel_table):
    global LAST_EXEC_NS, LAST_RES
    query = np.asarray(query, np.float32)
    key = np.asarray(key, np.float32)
    value = np.asarray(value, np.float32)
    attn_mask = np.asarray(attn_mask, bool)
    Wq, bq = np.asarray(Wq, np.float32), np.asarray(bq, np.float32)
    Wk, bk = np.asarray(Wk, np.float32), np.asarray(bk, np.float32)
    Wv, bv = np.asarray(Wv, np.float32), np.asarray(bv, np.float32)
    Wu, bu = np.asarray(Wu, np.float32), np.asarray(bu, np.float32)
    Wf2, bf2 = np.asarray(Wf2, np.float32), np.asarray(bf2, np.float32)
    rel_table = np.asarray(rel_table, np.float32)

    tril = np.tril(np.ones((S, S), bool))
    causal = all(np.array_equal(attn_mask[b], tril) for b in range(B))

    key_ = (causal, MM_DT, "v2")
    if key_ not in _CACHE:
        _CACHE[key_] = (_build_v2() if causal
                        else _build_legacy(causal))
    nc = _CACHE[key_]

    in_maps = _host_shards(query, key, value, attn_mask, Wq, bq, Wk, bk,
                           Wv, bv, Wu, bu, Wf2, rel_table, causal)
    res = run_bass_kernel_spmd(nc, in_maps, list(range(NCORES)), trace=TRACE)
    LAST_RES = res
    if res.exec_time_ns is not None:
        LAST_EXEC_NS = res.exec_time_ns

    outp = np.empty((B, S, H), np.float32)
    for b in range(B):
        outp[b] = (res.results[2 * b]["out"] + res.results[2 * b + 1]["out"]
                   + bf2[None, :])
    return outp
